# revision 12
# baseline (speedup 1.0000x reference)
"""BiDirectionalAddBlock (dual Mamba branches) on 8 TRN2 NeuronCores.

Sharding: core c = (batch b=c>>2, branch m=(c>>1)&1, d_inner half q=c&1).
Each core runs the full sequence for its (batch, branch): layernorm,
in-projection, conv+silu (both halves; the other half streams straight
into the dbc matmul), then the selective scan for its own 768-channel
half. Partial outputs are summed on the host (halves + branches).

Scan layout: partitions = (n=16 outer, d8=8 inner) per "octet" of 8
channels, t on the free dim. dA = Exp(selector-matmul(A-folded, delta)),
dBx = selector-matmul(ones, delta*xc) * Brep, recurrence via DVE
tensor_tensor_scan, n-contraction via PSUM-accumulated selector matmuls.
"""
import numpy as np
from contextlib import ExitStack

import concourse.bass as bass
import concourse.tile as tile
from concourse import mybir
from concourse.bass_utils import run_bass_kernel_spmd
from concourse.vector_clock import ScopedClock

F32 = mybir.dt.float32
BF16 = mybir.dt.bfloat16
ALU = mybir.AluOpType
AF = mybir.ActivationFunctionType

D_MODEL = 768
D_CONV = 4
D_INNER = 1536
DH = 768            # d_inner half per core
DT_RANK = 48
B, L_FULL = 2, 1024
ND = 6              # d-tiles of 128 in DH
POOL_HC = True      # route half the hC multiplies to GpSimd


# ---------------------------------------------------------------- tile patch
def _drain_and_barrier(self, tick_clock, wait_clock):
    """walrus rejects instructions with many sync waits ("Too many sync
    wait commands") on the final Drain; hoist waits onto NOP carriers."""
    nc = self.nc
    carrier = nc.sync.nop(nofuse=True)
    wait_clock.add_sem_waits(carrier.ins, ScopedClock({None: tick_clock.global_clock}))
    si = carrier.ins.sync_info
    waits = list(si.on_wait) if si is not None else []
    if len(waits) > 1:
        si.on_wait = waits[:1]
        for w in waits[1:]:
            extra = nc.sync.nop(nofuse=True)
            extra.ins.sync_info = mybir.SyncInfo(on_wait=[w], on_update=[])
    nc.sync.drain()
    nc.all_engine_barrier()
    assert self.sems is not None
    popped = nc._tile_sem_poison_stack.pop()
    assert popped is self._sem_poison
    nc.clear_and_free_semaphores(list(self.sems.allocated().values()))
    nc.all_engine_barrier()


tile.TileContext._drain_and_barrier = _drain_and_barrier

_OPCODE_MAXW = {"Matmult": 1, "DMACopy": 1}


def split_sync_waits(nc, default_maxw=1):
    """walrus codegen limits sync waits per instruction; hoist extras
    onto same-engine NOPs inserted immediately before the instruction."""
    nid = [0]
    for f in nc.m.functions:
        for bb in f.blocks:
            insts = list(bb.instructions)
            out, changed = [], False
            for inst in insts:
                si = inst.sync_info
                waits = list(si.on_wait) if si is not None else []
                maxw = _OPCODE_MAXW.get(type(inst).__name__.replace("Inst", ""),
                                        default_maxw)
                if len(waits) > maxw:
                    extra, keep = waits[:-maxw], waits[-maxw:]
                    for g0 in range(0, len(extra), maxw):
                        nid[0] += 1
                        nop = mybir.InstNoOp(
                            name=f"WSPLIT-{nid[0]}", engine=inst.engine,
                            sync_info=mybir.SyncInfo(
                                on_wait=extra[g0:g0 + maxw], on_update=[]))
                        nc.register_instruction(nop)
                        out.append(nop)
                    si.on_wait = keep
                    changed = True
                out.append(inst)
            if changed:
                bb.instructions = out


def _bcast_ap(ap, n):
    """Partition-broadcast AP: replicate a 1-partition (or partitionless)
    view across n partitions."""
    return bass.AP(tensor=ap.tensor, offset=ap.offset,
                   ap=[[0, n]] + [list(d) for d in ap.ap[1:]])


def _bcast_vec(ap, n):
    """Broadcast a 1-D dram vector across n partitions."""
    return bass.AP(tensor=ap.tensor, offset=ap.offset,
                   ap=[[0, n]] + [list(d) for d in ap.ap])


# ------------------------------------------------------------------- builder
def build_nc(L, ln_identity):
    NT = L // 128          # t-tiles
    NC2 = L // 512         # 512-col chunks
    nc = bass.Bass("TRN2", target_bir_lowering=False, debug=False, num_devices=8)

    def inp(name, shape, dt=F32):
        return nc.declare_dram_parameter(name, list(shape), dt, isOutput=False)

    xs = inp("xs", (L, D_MODEL))
    lng = inp("lng", (D_MODEL,))
    lnb = inp("lnb", (D_MODEL,))
    wxiT = inp("wxiT", (D_MODEL, D_INNER))      # cols in ord_d order
    wzT = inp("wzT", (D_MODEL, DH))
    cdiag = inp("cdiag", (12, D_CONV, 128, 128))
    convbT = inp("convbT", (128, 12))
    xpWT = inp("xpWT", (D_INNER, 80))           # rows in ord_d order
    dtWT = inp("dtWT", (DT_RANK, DH))
    dtbT = inp("dtbT", (128, ND))
    asel = inp("asel", (16 * ND, 128, 128))
    osel = inp("osel", (16, 128, 128), BF16)
    ssel = inp("ssel", (16, 128, 128))
    dskT = inp("dskT", (128, ND))
    outWT = inp("outWT", (DH, D_MODEL))
    eye = inp("eye", (128, 128))
    out = nc.declare_dram_parameter("out", [L, D_MODEL], F32, isOutput=True)

    def chunks(n0, n1, step=512):
        o = n0
        while o < n1:
            yield o, min(n1 - o, step)
            o += step

    with tile.TileContext(nc) as tc, ExitStack() as ctx:
        ep = ctx.enter_context

        consts = ep(tc.tile_pool(name="consts", bufs=1))
        eye_s = consts.tile([128, 128], F32)
        nc.sync.dma_start(out=eye_s, in_=eye[:])
        convb_s = consts.tile([128, 12], F32)
        nc.sync.dma_start(out=convb_s, in_=convbT[:])
        dtb_s = consts.tile([128, ND], F32)
        nc.sync.dma_start(out=dtb_s, in_=dtbT[:])
        ndtb_s = consts.tile([128, ND], F32)
        nc.scalar.mul(out=ndtb_s, in_=dtb_s, mul=-1.0)
        dsk_s = consts.tile([128, ND], F32)
        nc.sync.dma_start(out=dsk_s, in_=dskT[:])
        osel_s = consts.tile([128, 16 * 128], BF16)
        nc.sync.dma_start(out=osel_s.rearrange("p (o m) -> p o m", o=16),
                          in_=osel[:].rearrange("o p m -> p o m"))
        ssel_s = consts.tile([128, 16 * 128], F32)
        nc.sync.dma_start(out=ssel_s.rearrange("p (o m) -> p o m", o=16),
                          in_=ssel[:].rearrange("o p m -> p o m"))
        dtWT_s = consts.tile([DT_RANK, DH], F32)
        nc.sync.dma_start(out=dtWT_s, in_=dtWT[:])
        dbc_s = consts.tile([80, L], F32)

        big = ep(tc.tile_pool(name="big", bufs=1))
        szT = big.tile([128, ND * L], F32, tag="szT")
        deltaT = big.tile([128, ND * L], F32, tag="deltaT")
        dxT = big.tile([128, ND * L], BF16, tag="dxT")
        xcO = big.tile([128, ND * L], F32, tag="xcO")
        y3T = big.tile([128, ND * L], F32, tag="y3T")
        brep = big.tile([128, L], F32, tag="brep")
        crep = big.tile([128, L], F32, tag="crep")

        with tc.tile_pool(name="xnT_p", bufs=1) as xnp:
            xnT = xnp.tile([128, ND * L], F32)

            # ------------- phase A: layernorm + transpose -> xnT
            with tc.tile_pool(name="phA", bufs=3) as pa, \
                 tc.tile_pool(name="phA_ps", bufs=4, space="PSUM") as pat:
                eps_t = pa.tile([128, 1], F32, tag="eps")
                nc.vector.memset(eps_t, 1e-5)
                if not ln_identity:
                    g_t = pa.tile([128, D_MODEL], F32, tag="g")
                    nc.sync.dma_start(out=g_t, in_=_bcast_vec(lng[:], 128))
                    b_t = pa.tile([128, D_MODEL], F32, tag="b")
                    nc.sync.dma_start(out=b_t, in_=_bcast_vec(lnb[:], 128))
                for tt in range(NT):
                    x_t = pa.tile([128, D_MODEL], F32, tag="x")
                    nc.sync.dma_start(out=x_t, in_=xs[tt * 128:(tt + 1) * 128, :])
                    stats = pa.tile([128, 3, 6], F32, tag="stats")
                    xg = x_t.rearrange("p (s d) -> p s d", s=3)
                    for s in range(3):
                        nc.vector.bn_stats(out=stats[:, s, :], in_=xg[:, s, :])
                    mv = pa.tile([128, 2], F32, tag="mv")
                    nc.vector.bn_aggr(out=mv, in_=stats)
                    rstd = pa.tile([128, 1], F32, tag="rstd")
                    nc.scalar.activation(out=rstd, in_=mv[:, 1:2], func=AF.Sqrt,
                                         bias=eps_t, scale=1.0)
                    nc.vector.reciprocal(out=rstd, in_=rstd)
                    xn_t = pa.tile([128, D_MODEL], F32, tag="xn")
                    nc.vector.tensor_scalar(out=xn_t, in0=x_t, scalar1=mv[:, 0:1],
                                            scalar2=rstd, op0=ALU.subtract,
                                            op1=ALU.mult)
                    if not ln_identity:
                        nc.vector.tensor_mul(xn_t, xn_t, g_t)
                        nc.vector.tensor_add(xn_t, xn_t, b_t)
                    for D in range(ND):
                        pt = pat.tile([128, 128], F32)
                        nc.tensor.transpose(pt, xn_t[:, D * 128:(D + 1) * 128],
                                            eye_s)
                        nc.scalar.copy(
                            out=xnT[:, D * L + tt * 128: D * L + (tt + 1) * 128],
                            in_=pt)

            # ------------- phases B-E: z, xi+conv (streamed), dbc, delta
            with tc.tile_pool(name="wstrip", bufs=1) as wsp, \
                 tc.tile_pool(name="phB_ps", bufs=2, space="PSUM") as pbp, \
                 tc.tile_pool(name="dbc_ps_p", bufs=1, space="PSUM") as pdbc:
                # z -> silu(z)
                with tc.tile_pool(name="wz_p", bufs=1) as wzp:
                  wz_s = wzp.tile([128, ND * 128 * 6], F32, tag="wz")
                  wz2 = wz_s.rearrange("p (k m) -> p k m", k=6)
                  for k in range(6):
                    nc.sync.dma_start(out=wz2[:, k, :],
                                      in_=wzT[k * 128:(k + 1) * 128, :])
                  for mj in range(ND):
                    ps = pbp.tile([128, L], F32, tag="ps")
                    for co, cw in chunks(0, L):
                        for k in range(6):
                            nc.tensor.matmul(
                                ps[:, co:co + cw],
                                lhsT=wz2[:, k, mj * 128:(mj + 1) * 128],
                                rhs=xnT[:, k * L + co: k * L + co + cw],
                                start=(k == 0), stop=(k == 5))
                    nc.scalar.activation(out=szT[:, mj * L:(mj + 1) * L],
                                         in_=ps, func=AF.Silu)
                  del wz_s, wz2

                dbc_psum = pdbc.tile([80, L], F32)
                xpw_s = wsp.tile([128, 12 * 80], F32, tag="xpw")
                xpw2 = xpw_s.rearrange("p (k m) -> p k m", k=12)
                for k in range(12):
                    nc.sync.dma_start(out=xpw2[:, k, :],
                                      in_=xpWT[k * 128:(k + 1) * 128, :])

                for half in range(2):     # 0 = own, 1 = other
                    with tc.tile_pool(name=f"wxi_{half}", bufs=1) as wxp, \
                         tc.tile_pool(name=f"xis_{half}", bufs=2) as xip, \
                         tc.tile_pool(name=f"xcs_{half}", bufs=2) as xcs:
                        wxi_s = wxp.tile([128, 6 * DH], F32)
                        wxi2 = wxi_s.rearrange("p (k m) -> p k m", k=6)
                        for k in range(6):
                            nc.sync.dma_start(
                                out=wxi2[:, k, :],
                                in_=wxiT[k * 128:(k + 1) * 128,
                                         half * DH:(half + 1) * DH])
                        for mj in range(ND):
                            gt = half * ND + mj     # global xc tile in ord_d
                            ps = pbp.tile([128, L], F32, tag="ps")
                            for co, cw in chunks(0, L):
                                for k in range(6):
                                    nc.tensor.matmul(
                                        ps[:, co:co + cw],
                                        lhsT=wxi2[:, k, mj * 128:(mj + 1) * 128],
                                        rhs=xnT[:, k * L + co: k * L + co + cw],
                                        start=(k == 0), stop=(k == 5))
                            xi_t = xip.tile([128, L], F32, tag="xi")
                            nc.scalar.copy(out=xi_t, in_=ps)
                            pc = pbp.tile([128, L], F32, tag="ps")
                            cd = wxp.tile([128, 4 * 128], F32, tag="cd")
                            cd2 = cd.rearrange("p (k m) -> p k m", k=4)
                            for k in range(4):
                                nc.sync.dma_start(out=cd2[:, k, :],
                                                  in_=cdiag[gt, k, :, :])
                            for co, cw in chunks(0, L):
                                nc.tensor.matmul(pc[:, co:co + cw],
                                                 lhsT=cd2[:, 3, :],
                                                 rhs=xi_t[:, co:co + cw],
                                                 start=True, stop=False,
                                                 skip_group_check=True)
                            for k in range(3):
                                s = 3 - k
                                for co, cw in chunks(s, L):
                                    nc.tensor.matmul(
                                        pc[:, co:co + cw], lhsT=cd2[:, k, :],
                                        rhs=xi_t[:, co - s:co - s + cw],
                                        start=False,
                                        stop=(k == 2 and co + cw >= L),
                                        skip_group_check=True)
                            if half == 0:
                                xct = xcO[:, mj * L:(mj + 1) * L]
                            else:
                                xct = xcs.tile([128, L], F32, tag="xc")
                            nc.scalar.activation(out=xct, in_=pc, func=AF.Silu,
                                                 bias=convb_s[:, gt:gt + 1],
                                                 scale=1.0)
                            for co, cw in chunks(0, L):
                                nc.tensor.matmul(
                                    dbc_psum[:, co:co + cw], lhsT=xpw2[:, gt, :],
                                    rhs=xct[:, co:co + cw],
                                    start=(gt == 0), stop=(gt == 11),
                                    skip_group_check=True)
                nc.scalar.copy(out=dbc_s, in_=dbc_psum)
                # delta = softplus(dtp @ dtW.T + dtb); dx = delta * xc_own
                for D in range(ND):
                    pd = pbp.tile([128, L], F32, tag="ps")
                    for co, cw in chunks(0, L):
                        nc.tensor.matmul(pd[:, co:co + cw],
                                         lhsT=dtWT_s[:, D * 128:(D + 1) * 128],
                                         rhs=dbc_s[0:DT_RANK, co:co + cw],
                                         start=True, stop=True)
                    # softplus(u) = u + ln(1 + exp(-u)), u = pd + dtb
                    e_t = wsp.tile([128, L], F32, tag="spe")
                    nc.scalar.activation(out=e_t, in_=pd, func=AF.Exp,
                                         scale=-1.0, bias=ndtb_s[:, D:D + 1])
                    l_t = wsp.tile([128, L], F32, tag="spl")
                    nc.scalar.activation(out=l_t, in_=e_t, func=AF.Ln, bias=1.0)
                    nc.vector.scalar_tensor_tensor(
                        out=deltaT[:, D * L:(D + 1) * L], in0=pd,
                        scalar=dtb_s[:, D:D + 1], in1=l_t,
                        op0=ALU.add, op1=ALU.add)
                    nc.vector.tensor_mul(dxT[:, D * L:(D + 1) * L],
                                         deltaT[:, D * L:(D + 1) * L],
                                         xcO[:, D * L:(D + 1) * L])
            bc_dram = nc.dram_tensor("bc_scratch", [32, L], F32)
            nc.sync.dma_start(out=bc_dram[:], in_=dbc_s[DT_RANK:80, :])
            for n in range(16):
                nc.sync.dma_start(out=brep[n * 8:(n + 1) * 8, :],
                                  in_=_bcast_vec(bc_dram[n, :], 8))
                nc.sync.dma_start(out=crep[n * 8:(n + 1) * 8, :],
                                  in_=_bcast_vec(bc_dram[16 + n, :], 8))

            # ------------- scan phase
            with tc.tile_pool(name="sc", bufs=2) as scp, \
                 tc.tile_pool(name="scA_ps", bufs=2, space="PSUM") as pda, \
                 tc.tile_pool(name="scX_ps", bufs=1, space="PSUM") as pdx, \
                 tc.tile_pool(name="scY_ps", bufs=1, space="PSUM") as pyt:
                for D in range(ND):
                    yt = pyt.tile([128, L], F32, tag="yt")
                    for o in range(16):
                        j = 16 * D + o
                        a_t = scp.tile([128, 128], F32, tag="asel")
                        nc.sync.dma_start(out=a_t, in_=asel[j, :, :])
                        pA = pda.tile([128, L], F32, tag="pA")
                        pX = pdx.tile([128, L], F32, tag="pX")
                        for co, cw in chunks(0, L):
                            nc.tensor.matmul(
                                pA[:, co:co + cw], lhsT=a_t,
                                rhs=deltaT[:, D * L + co: D * L + co + cw],
                                start=True, stop=True)
                            nc.tensor.matmul(
                                pX[:, co:co + cw],
                                lhsT=osel_s[:, o * 128:(o + 1) * 128],
                                rhs=dxT[:, D * L + co: D * L + co + cw],
                                start=True, stop=True)
                        dA = scp.tile([128, L], F32, tag="dA")
                        nc.scalar.activation(out=dA, in_=pA, func=AF.Exp)
                        dBx = scp.tile([128, L], F32, tag="dBx")
                        nc.vector.scalar_tensor_tensor(
                            out=dBx, in0=pX, scalar=1.0, in1=brep,
                            op0=ALU.mult, op1=ALU.mult)
                        h = scp.tile([128, L], F32, tag="h")
                        nc.vector.tensor_tensor_scan(
                            out=h, data0=dA, data1=dBx, initial=0.0,
                            op0=ALU.mult, op1=ALU.add)
                        hC = scp.tile([128, L], F32, tag="hC")
                        eng = nc.gpsimd if (POOL_HC and o % 2 == 1) else nc.vector
                        eng.tensor_mul(hC, h, crep)
                        for co, cw in chunks(0, L):
                            nc.tensor.matmul(
                                yt[:, co:co + cw],
                                lhsT=ssel_s[:, o * 128:(o + 1) * 128],
                                rhs=hC[:, co:co + cw],
                                start=(o == 0), stop=(o == 15))
                    tmp = scp.tile([128, L], F32, tag="tmp")
                    nc.vector.scalar_tensor_tensor(
                        out=tmp, in0=xcO[:, D * L:(D + 1) * L],
                        scalar=dsk_s[:, D:D + 1], in1=yt,
                        op0=ALU.mult, op1=ALU.add)
                    nc.vector.tensor_mul(y3T[:, D * L:(D + 1) * L], tmp,
                                         szT[:, D * L:(D + 1) * L])

        # ------------- output matmul: out[t, :] = y3T.T @ outWT
        with tc.tile_pool(name="ow", bufs=1) as owp, \
             tc.tile_pool(name="ob", bufs=3) as obp, \
             tc.tile_pool(name="out_ps", bufs=2, space="PSUM") as pop:
            ow_s = owp.tile([128, ND * D_MODEL], F32)
            ow2 = ow_s.rearrange("p (k m) -> p k m", k=ND)
            for k in range(ND):
                nc.sync.dma_start(out=ow2[:, k, :],
                                  in_=outWT[k * 128:(k + 1) * 128, :])
            for tt in range(NT):
                po = pop.tile([128, D_MODEL], F32, tag="po")
                for co, cw in chunks(0, D_MODEL):
                    for k in range(ND):
                        nc.tensor.matmul(
                            po[:, co:co + cw],
                            lhsT=y3T[:, k * L + tt * 128: k * L + (tt + 1) * 128],
                            rhs=ow2[:, k, co:co + cw],
                            start=(k == 0), stop=(k == ND - 1))
                ob = obp.tile([128, D_MODEL], F32, tag="ob")
                nc.scalar.copy(out=ob, in_=po)
                nc.sync.dma_start(out=out[tt * 128:(tt + 1) * 128, :], in_=ob)

    split_sync_waits(nc)
    return nc


# ------------------------------------------------------------------- host
def make_core_inputs(x_seq, ln_g, ln_b, p, q):
    """Per-core input map. p = dict of branch params, q = d_inner half."""
    sl = slice(q * DH, (q + 1) * DH)
    A_q = -np.exp(np.asarray(p["Alog"], np.float64))[sl].astype(np.float32)
    conv_w = np.asarray(p["convw"], np.float32)

    asel = np.zeros((16 * ND, 128, 128), np.float32)
    i_idx = np.arange(8)
    n_idx = np.arange(16)
    for j in range(16 * ND):
        o = j % 16
        # rows 8o+i, cols n*8+i  <- A_q[8j+i, n]
        asel[j, (8 * o + i_idx)[None, :], (8 * n_idx[:, None] + i_idx[None, :])] = \
            A_q[8 * j + i_idx[None, :], n_idx[:, None]]
    osel = np.zeros((16, 128, 128), np.float32)
    ssel = np.zeros((16, 128, 128), np.float32)
    for o in range(16):
        osel[o, (8 * o + i_idx)[None, :], 8 * n_idx[:, None] + i_idx[None, :]] = 1.0
        ssel[o, 8 * n_idx[:, None] + i_idx[None, :], (8 * o + i_idx)[None, :]] = 1.0

    # xc tile order: own half first (tiles 0..5), other half after (6..11)
    ord_d = np.r_[np.arange(q * DH, (q + 1) * DH),
                  np.arange((1 - q) * DH, (2 - q) * DH)]
    cdiag = np.zeros((12, D_CONV, 128, 128), np.float32)
    for Dt in range(12):
        rows = ord_d[Dt * 128:(Dt + 1) * 128]
        for k in range(D_CONV):
            np.fill_diagonal(cdiag[Dt, k], conv_w[rows, k])
    convbT = np.asarray(p["convb"], np.float32)[ord_d].reshape(12, 128).T

    c = lambda a: np.ascontiguousarray(a, np.float32)
    inW = np.asarray(p["inW"], np.float32)
    return {
        "xs": c(x_seq),
        "lng": c(ln_g), "lnb": c(ln_b),
        "wxiT": c(inW[:D_INNER][ord_d].T),
        "wzT": c(inW[D_INNER + q * DH: D_INNER + (q + 1) * DH].T),
        "cdiag": cdiag,
        "convbT": c(convbT),
        "xpWT": c(np.asarray(p["xpW"], np.float32)[:, ord_d].T),
        "dtWT": c(np.asarray(p["dtW"], np.float32)[sl].T),
        "dtbT": c(np.asarray(p["dtb"], np.float32)[sl].reshape(ND, 128).T),
        "asel": asel,
        "osel": osel.astype(np.float32),  # cast to bf16 by runner via dtype? no:
        "ssel": ssel,
        "dskT": c(np.asarray(p["Dsk"], np.float32)[sl].reshape(ND, 128).T),
        "outWT": c(np.asarray(p["outW"], np.float32)[:, sl].T),
        "eye": np.eye(128, dtype=np.float32),
    }


_NC_CACHE = {}


def kernel(**inputs):
    import ml_dtypes
    x = np.asarray(inputs["x"], np.float32)
    ln_g = np.asarray(inputs["ln_g"], np.float32)
    ln_b = np.asarray(inputs["ln_b"], np.float32)
    params = {}
    for tag in ("m1", "m2"):
        params[tag] = {k.split("_", 1)[1]: np.asarray(v)
                       for k, v in inputs.items() if k.startswith(tag + "_")}

    ln_identity = bool(np.all(ln_g == 1.0) and np.all(ln_b == 0.0))
    key = (L_FULL, ln_identity)
    if key not in _NC_CACHE:
        _NC_CACHE[key] = build_nc(L_FULL, ln_identity)
    nc = _NC_CACHE[key]

    in_maps = []
    for c in range(8):
        b, m, q = c >> 2, (c >> 1) & 1, c & 1
        x_seq = x[b] if m == 0 else np.ascontiguousarray(x[b, ::-1])
        im = make_core_inputs(x_seq, ln_g, ln_b,
                              params["m1" if m == 0 else "m2"], q)
        im["osel"] = im["osel"].astype(ml_dtypes.bfloat16)
        in_maps.append(im)

    res = run_bass_kernel_spmd(nc, in_maps, core_ids=list(range(8)))
    hidden = np.zeros((B, L_FULL, D_MODEL), np.float32)
    for c in range(8):
        hidden[c >> 2] += res.results[c]["out"]
    return hidden, x
